# revision 16
# baseline (speedup 1.0000x reference)
"""Trainium2 Bass kernel for nn_MixtureAlignmentLogLikelihood.

Math: with trg_p = softmax(trg_sent, axis=2), every row of trg_p sums to 1
and P_st is the uniform matrix 1/Kt, so dot[b,t] == 1/Kt exactly and

  log_likelihood = -log(Kt) * sum(scales)

sum(scales) depends only on trg_boundary: per batch row (T positions,
boundary bits z):

  count = popcount(z); first = z[0]; lastp1 = (last set index)+1 (0 if none)
  sum_scales = count - first - max(lastp1, 1) + T + 1

Device kernel (per core): 32 batch rows, each row's T=2048 split into 4
quarters of 512 -> SBUF [128, 512] int8 (row b = partitions 4b..4b+3),
using all 128 partitions (4x the baseline's partition utilization). The
engine streams are emitted WITHOUT nc.Block() (explicit semaphores carry
all cross-engine deps) - the Block-exit barrier costs ~0.7us extra.

  SP  : DMA tb -> SBUF (single HWDGE queue; issuing DMAs from the ACT
        engine's queue or gpsimd SW-DGE inflates the NEFF's fixed
        semaphore-reset epilogue by 5-15us, measured, so everything stays
        on SP); final DMA pack[128,2] -> out
  ACT : dummy act first (pulls the 1.28us ACT_TABLE_LOAD into the
        DMA-flight window), then count = add-accum(Copy(tb)) -> pack[:,0]
  Pool: iot = iota(1..512) on all 128 partitions (parallel with the DMA)
  DVE : prod = tb*iot; pack[:,1] = max-pool(prod), f32 out (int16 pool
        output computes WRONG results on HW; fused mul+reduce and
        scalar_tensor_tensor are rejected by this walrus build)

Per-quarter stats are combined on the host during the gather (the same
place the baseline summed its per-row outputs): per row,
  count = sum_q count_q;  lastp1 = max_q (512*q*[qmax_q>0] + qmax_q)
  first = trg_boundary[b, 0]   (host already holds the input)
All quantities are small integers -> exact in int16/f32. Batch is sharded
32 rows per core across 8 NeuronCores (pure data parallel); per-core row
log-likelihoods are summed on the host (the scalar all-reduce). The final
output DMA is not engine-waited: NEFF completion semantics cover it
(verified empirically on the baseline over repeated randomized runs).
"""

import math

import numpy as np

B, T, K = 256, 2048, 64
N_CORES = 8
BS = B // N_CORES  # 32 batch rows per core
Q = 4  # quarters per row
S = T // Q  # 512 positions per quarter
P = BS * Q  # 128 partitions
NEG_LOG_K = -math.log(float(K))

_CACHE: dict = {}


def _build_nc(
    final_wait: bool = False,
    in_i8: bool = True,
    adum_i16: bool = True,
    use_pool_max: bool = True,
):
    import concourse.bass as bass
    import concourse.mybir as mybir

    f32 = mybir.dt.float32
    i16 = mybir.dt.int16
    i8 = mybir.dt.int8
    in_dt = i8 if in_i8 else i16

    nc = bass.Bass(enable_partition_id=False)
    tb = nc.dram_tensor("tb", [P, S], in_dt, kind="ExternalInput")
    out = nc.dram_tensor("out", [P, 2], f32, kind="ExternalOutput")

    with (
        nc.sbuf_tensor("tbs", [P, S], in_dt) as tbs,
        nc.sbuf_tensor("iot", [P, S], i16) as iot,
        nc.sbuf_tensor("prod", [P, S], i16) as prod,
        nc.sbuf_tensor("adum", [P, S], i16 if adum_i16 else f32) as adum,
        nc.sbuf_tensor("pack", [P, 2], f32) as pack,
        nc.sbuf_tensor("scr", [P, 1], f32) as scr,
        nc.semaphore("dma_s") as dma_s,
        nc.semaphore("p_sem") as p_sem,
        nc.semaphore("a_sem") as a_sem,
        nc.semaphore("m_sem") as m_sem,
    ):
        const0 = nc.const_aps.aps[(f32, 0.0)]

        # Pool: index ramp, runs concurrently with the input DMA
        nc.gpsimd.iota(
            iot[:, :], pattern=[[1, S]], base=1, channel_multiplier=0
        ).then_inc(p_sem, 1)

        # SP: input DMA, then the single packed-output DMA. Wait on m_sem
        # (DVE, always last by ~400ns) first so a_sem is a zero-cost check
        # instead of a second engine wakeup.
        nc.sync.dma_start(tbs[:], tb[:, :]).then_inc(dma_s, 16)
        nc.sync.wait_ge(m_sem, 1)
        nc.sync.wait_ge(a_sem, 1)
        nc.sync.dma_start(out[:, :], pack[:]).then_inc(dma_s, 16)
        if final_wait:
            nc.sync.wait_ge(dma_s, 32)

        # ACT: dummy act pulls ACT_TABLE_LOAD into the DMA-flight window,
        # then count = add-accum(Copy(tb)); f32 accum of 0/1 ints is exact
        nc.scalar.activation(scr[:], const0, mybir.ActivationFunctionType.Copy)
        nc.scalar.wait_ge(dma_s, 16)
        nc.scalar.activation(
            adum[:],
            tbs[:],
            mybir.ActivationFunctionType.Copy,
            accum_out=pack[:, 0:1],
        ).then_inc(a_sem, 1)

        # DVE: prod = iot*tb; pack[:,1] = per-quarter max (0 = no bits set).
        # i16 operand first: with the i8 operand first the DVE loses its
        # 16-bit dual-pump (691ns vs 426ns measured).
        nc.vector.wait_ge(p_sem, 1)
        nc.vector.wait_ge(dma_s, 16)
        nc.vector.tensor_mul(prod[:], iot[:], tbs[:])
        if use_pool_max:
            nc.vector.pool_max(pack[:, 1:2], prod[:]).then_inc(m_sem, 1)
        else:
            nc.vector.reduce_max(
                pack[:, 1:2], prod[:], axis=mybir.AxisListType.X
            ).then_inc(m_sem, 1)

    return nc


def _get_nc(**kwargs):
    key = tuple(sorted(kwargs.items()))
    if key not in _CACHE:
        _CACHE[key] = _build_nc(**kwargs)
    return _CACHE[key]


def _in_maps(trg_boundary: np.ndarray, np_dt):
    tb = np.asarray(trg_boundary)
    assert tb.shape == (B, T), tb.shape
    tbn = tb.astype(np_dt)  # values are 0/1
    return [
        {"tb": np.ascontiguousarray(tbn[c * BS : (c + 1) * BS].reshape(P, S))}
        for c in range(N_CORES)
    ]


def run_device(trg_boundary, nc_kwargs=None, **run_kwargs):
    """Compile (cached) + run on cores 0-7; returns BassKernelResults."""
    from concourse.bass_utils import run_bass_kernel_spmd

    nck = nc_kwargs or {}
    np_dt = np.int8 if nck.get("in_i8", True) else np.int16
    return run_bass_kernel_spmd(
        _get_nc(**nck),
        _in_maps(trg_boundary, np_dt),
        core_ids=list(range(N_CORES)),
        **run_kwargs,
    )


def kernel(src_sent, trg_sent, src_boundary, trg_boundary):
    res = run_device(trg_boundary)
    tbf = np.asarray(trg_boundary)
    total = np.float64(0.0)
    qoff = np.float64(S) * np.arange(Q, dtype=np.float64)
    for c, r in enumerate(res.results):
        pack = np.asarray(r["out"], dtype=np.float64)  # [128, 2]
        count = pack[:, 0].reshape(BS, Q).sum(axis=1)
        qm = pack[:, 1].reshape(BS, Q)
        lastp1 = np.max(np.where(qm > 0, qoff[None, :] + qm, 0.0), axis=1)
        first = tbf[c * BS : (c + 1) * BS, 0].astype(np.float64)
        rows = count - first - np.maximum(lastp1, 1.0) + (T + 1)
        total += rows.sum() * NEG_LOG_K
    return np.asarray(total, dtype=np.float32)


# revision 17
# speedup vs baseline: 1.1198x; 1.1198x over previous
"""Trainium2 Bass kernel for nn_MixtureAlignmentLogLikelihood.

Math: with trg_p = softmax(trg_sent, axis=2), every row of trg_p sums to 1
and P_st is the uniform matrix 1/Kt, so dot[b,t] == 1/Kt exactly and

  log_likelihood = -log(Kt) * sum(scales)

sum(scales) depends only on trg_boundary: per batch row (T positions,
boundary bits z):

  count = popcount(z); first = z[0]; lastp1 = (last set index)+1 (0 if none)
  sum_scales = count - first - max(lastp1, 1) + T + 1

Device kernel (per core): 32 batch rows, each row's T=2048 split into 4
quarters of 512 -> SBUF [128, 512] int8 (row b = partitions 4b..4b+3),
using all 128 partitions (4x the baseline's partition utilization). The
engine streams are emitted WITHOUT nc.Block() (explicit semaphores carry
all cross-engine deps) - the Block-exit barrier costs ~0.7us extra.

  SP  : DMA tb -> SBUF (single HWDGE queue; issuing DMAs from the ACT
        engine's queue or gpsimd SW-DGE inflates the NEFF's fixed
        semaphore-reset epilogue by 5-15us, measured, so everything stays
        on SP); final DMA pack[128,2] -> out
  ACT : dummy act first (pulls the 1.28us ACT_TABLE_LOAD into the
        DMA-flight window), then count = add-accum(Copy(tb)) -> pack[:,0]
  Pool: iot = iota(1..512) on all 128 partitions (parallel with the DMA)
  DVE : prod = tb*iot; pack[:,1] = max-pool(prod), f32 out (int16 pool
        output computes WRONG results on HW; fused mul+reduce and
        scalar_tensor_tensor are rejected by this walrus build)

Per-quarter stats are combined on the host during the gather (the same
place the baseline summed its per-row outputs): per row,
  count = sum_q count_q;  lastp1 = max_q (512*q*[qmax_q>0] + qmax_q)
  first = trg_boundary[b, 0]   (host already holds the input)
All quantities are small integers -> exact in int16/f32. Batch is sharded
32 rows per core across 8 NeuronCores (pure data parallel); per-core row
log-likelihoods are summed on the host (the scalar all-reduce). The final
output DMA is not engine-waited: NEFF completion semantics cover it
(verified empirically on the baseline over repeated randomized runs).
"""

import math

import numpy as np

B, T, K = 256, 2048, 64
N_CORES = 8
BS = B // N_CORES  # 32 batch rows per core
Q = 4  # quarters per row
S = T // Q  # 512 positions per quarter
P = BS * Q  # 128 partitions
NEG_LOG_K = -math.log(float(K))

_CACHE: dict = {}


def _build_nc(
    final_wait: bool = False,
    in_i8: bool = True,
    adum_i16: bool = True,
    use_pool_max: bool = True,
):
    import concourse.bass as bass
    import concourse.mybir as mybir

    f32 = mybir.dt.float32
    i16 = mybir.dt.int16
    i8 = mybir.dt.int8
    in_dt = i8 if in_i8 else i16

    nc = bass.Bass(enable_partition_id=False)
    tb = nc.dram_tensor("tb", [P, S], in_dt, kind="ExternalInput")
    out = nc.dram_tensor("out", [P, 2], f32, kind="ExternalOutput")

    with (
        nc.sbuf_tensor("tbs", [P, S], in_dt) as tbs,
        nc.sbuf_tensor("iot", [P, S], i16) as iot,
        nc.sbuf_tensor("prod", [P, S], i16) as prod,
        nc.sbuf_tensor("adum", [P, S], i16 if adum_i16 else f32) as adum,
        nc.sbuf_tensor("pack", [P, 2], f32) as pack,
        nc.sbuf_tensor("scr", [P, 1], f32) as scr,
        nc.semaphore("dma_s") as dma_s,
        nc.semaphore("p_sem") as p_sem,
        nc.semaphore("a_sem") as a_sem,
        nc.semaphore("m_sem") as m_sem,
    ):
        const0 = nc.const_aps.aps[(f32, 0.0)]

        # Pool: index ramp, runs concurrently with the input DMA
        nc.gpsimd.iota(
            iot[:, :], pattern=[[1, S]], base=1, channel_multiplier=0
        ).then_inc(p_sem, 1)

        # SP: input DMA, then the single packed-output DMA. Wait on m_sem
        # (DVE, always last by ~400ns) first so a_sem is a zero-cost check
        # instead of a second engine wakeup.
        nc.sync.dma_start(tbs[:], tb[:, :]).then_inc(dma_s, 16)
        nc.sync.wait_ge(m_sem, 1)
        nc.sync.wait_ge(a_sem, 1)
        nc.sync.dma_start(out[:, :], pack[:]).then_inc(dma_s, 16)
        if final_wait:
            nc.sync.wait_ge(dma_s, 32)

        # ACT: dummy act pulls ACT_TABLE_LOAD into the DMA-flight window,
        # then count = add-accum(Copy(tb)); f32 accum of 0/1 ints is exact
        nc.scalar.activation(scr[:], const0, mybir.ActivationFunctionType.Copy)
        nc.scalar.wait_ge(dma_s, 16)
        nc.scalar.activation(
            adum[:],
            tbs[:],
            mybir.ActivationFunctionType.Copy,
            accum_out=pack[:, 0:1],
        ).then_inc(a_sem, 1)

        # DVE: prod = tb*iot; pack[:,1] = per-quarter max (0 = no bits set).
        # The mixed i8xi16 multiply runs at 691ns vs 426 for i16xi16
        # (operand order does not matter; measured both ways) - offset by
        # the i8 input halving the DMA transfer time.
        nc.vector.wait_ge(p_sem, 1)
        nc.vector.wait_ge(dma_s, 16)
        nc.vector.tensor_mul(prod[:], tbs[:], iot[:])
        if use_pool_max:
            nc.vector.pool_max(pack[:, 1:2], prod[:]).then_inc(m_sem, 1)
        else:
            nc.vector.reduce_max(
                pack[:, 1:2], prod[:], axis=mybir.AxisListType.X
            ).then_inc(m_sem, 1)

    return nc


def _get_nc(**kwargs):
    key = tuple(sorted(kwargs.items()))
    if key not in _CACHE:
        _CACHE[key] = _build_nc(**kwargs)
    return _CACHE[key]


def _in_maps(trg_boundary: np.ndarray, np_dt):
    tb = np.asarray(trg_boundary)
    assert tb.shape == (B, T), tb.shape
    tbn = tb.astype(np_dt)  # values are 0/1
    return [
        {"tb": np.ascontiguousarray(tbn[c * BS : (c + 1) * BS].reshape(P, S))}
        for c in range(N_CORES)
    ]


def run_device(trg_boundary, nc_kwargs=None, **run_kwargs):
    """Compile (cached) + run on cores 0-7; returns BassKernelResults."""
    from concourse.bass_utils import run_bass_kernel_spmd

    nck = nc_kwargs or {}
    np_dt = np.int8 if nck.get("in_i8", True) else np.int16
    return run_bass_kernel_spmd(
        _get_nc(**nck),
        _in_maps(trg_boundary, np_dt),
        core_ids=list(range(N_CORES)),
        **run_kwargs,
    )


def kernel(src_sent, trg_sent, src_boundary, trg_boundary):
    res = run_device(trg_boundary)
    tbf = np.asarray(trg_boundary)
    total = np.float64(0.0)
    qoff = np.float64(S) * np.arange(Q, dtype=np.float64)
    for c, r in enumerate(res.results):
        pack = np.asarray(r["out"], dtype=np.float64)  # [128, 2]
        count = pack[:, 0].reshape(BS, Q).sum(axis=1)
        qm = pack[:, 1].reshape(BS, Q)
        lastp1 = np.max(np.where(qm > 0, qoff[None, :] + qm, 0.0), axis=1)
        first = tbf[c * BS : (c + 1) * BS, 0].astype(np.float64)
        rows = count - first - np.maximum(lastp1, 1.0) + (T + 1)
        total += rows.sum() * NEG_LOG_K
    return np.asarray(total, dtype=np.float32)
